# revision 37
# baseline (speedup 1.0000x reference)
"""Multi-head attention (16 heads, RoPE, causal) Trainium2 Bass kernel.

Sharding: 8 cores = 4-way data-parallel over batch x 2-way tensor-parallel
over heads (each core: 1 batch, 8 heads). Per-core partial outputs (over its
8 heads) are summed pairwise on the host (the w_o "all-reduce").

v2: all-bf16 datapath. All matmul operands are bf16 (fast weight load, no
f32r small-moving penalty, half the input DMA bytes). The softmax rowsum is
computed by pre-accumulating the 8 P^T k-tiles on the Vector engine (bf16
SBUF adds run 2 elem/cycle) into one [128, S] tile, then a single ones-
matmul per head reduces over partitions -- replacing 4608 PE matmul columns
per head with 1024. Elementwise work (RoPE multiplies/adds, normalization,
reciprocal, v copies) is statically balanced across Vector / GpSimd /
Scalar so no engine exceeds the PE's matmul stream.

Per-core algorithm (S=1024, E=128 = head dim, 8 local heads):
  - xT [e, s] bf16; per-head wT [e, d] blocks give qT/kT in [d, s] layout.
  - RoPE: rot(q)T = ropeC (.) qT + ropeS (.) (perm q)T with perm via
    pair-swapped weight copies (extra projection matmuls).
  - S^T[k, q] blocks per 128-wide k tile; causal diagonal gets a -1e30
    upper-triangular bias via one bf16 idn x tri matmul into the same PSUM
    accumulation group; Scalar applies exp(scale*x) writing P^T bf16.
  - rowsums: DVE pre-sum of P^T tiles + one ones-matmul; fast reciprocal.
  - y^T[d, q] = sum_j v_j @ P^T_j, normalized by recip rowsums, then
    out^T[e, s] += woT_h.T @ ynT_h accumulated in PSUM across heads.
"""

import os
import sys

import ml_dtypes
import numpy as np

for _p in ("/opt/trn_rl_repo",):
    if os.path.isdir(_p) and _p not in sys.path:
        sys.path.append(_p)

import concourse.bass as bass  # noqa: E402
import concourse.tile as tile  # noqa: E402
from concourse import bacc, mybir  # noqa: E402
from concourse.bass_utils import run_bass_kernel_spmd  # noqa: E402

F32 = mybir.dt.float32
BF16 = mybir.dt.bfloat16

B, S, E, H = 4, 1024, 128, 16
NCORES = 8
NH = 8          # heads per core
P = 128
NT = S // P     # 8 seq tiles
SCALE = 1.0 / float(np.sqrt(np.float32(E)))
Exp = mybir.ActivationFunctionType.Exp
MULT = mybir.AluOpType.mult
ADD = mybir.AluOpType.add


def build_bass():
    nc = bacc.Bacc("TRN2", target_bir_lowering=False, debug=False,
                   num_devices=NCORES)

    def din(name, shape, dt=BF16):
        return nc.dram_tensor(name, shape, dt, kind="ExternalInput").ap()

    xT = din("xT", [P, S])
    wqT = din("wqT", [P, NH * P])
    wqpT = din("wqpT", [P, NH * P])
    wkT = din("wkT", [P, NH * P])
    wkpT = din("wkpT", [P, NH * P])
    wvT = din("wvT", [P, NH * P])
    woT = din("woT", [P, NH * P])
    ropeCS = din("ropeCS", [P, 2 * S])
    tri = din("tri", [P, P])
    idn = din("idn", [P, P])
    ones = din("ones", [P, P])
    outT = nc.dram_tensor("outT", [P, S], F32, kind="ExternalOutput").ap()

    with tile.TileContext(nc) as tc:
        _build(tc, xT, wqT, wqpT, wkT, wkpT, wvT, woT, ropeCS, tri,
               idn, ones, outT)
    nc.compile()
    return nc


def _build(tc, xT, wqT, wqpT, wkT, wkpT, wvT, woT, ropeCS, tri, idn,
           ones, outT):
    nc = tc.nc

    from contextlib import ExitStack
    ctx = ExitStack()
    const = ctx.enter_context(tc.tile_pool(name="const", bufs=1))
    vpool = ctx.enter_context(tc.tile_pool(name="vpool", bufs=1))
    ppool = ctx.enter_context(tc.tile_pool(name="ppool", bufs=2))
    qkpool = ctx.enter_context(tc.tile_pool(name="qkpool", bufs=2))
    tmppool = ctx.enter_context(tc.tile_pool(name="tmppool", bufs=2))
    npool = ctx.enter_context(tc.tile_pool(name="npool", bufs=2))
    opool = ctx.enter_context(tc.tile_pool(name="opool", bufs=1))
    # PSUM budget (8 banks): proj 2-bank pair, S^T ring 3, one rotating
    # bank for the rowsum/AV accumulation groups, outproj accumulators 2.
    pp = ctx.enter_context(tc.tile_pool(name="pp", bufs=1, space="PSUM"))
    sp = ctx.enter_context(tc.tile_pool(name="sp", bufs=3, space="PSUM"))
    ap_ = ctx.enter_context(tc.tile_pool(name="ap", bufs=1, space="PSUM"))
    op = ctx.enter_context(tc.tile_pool(name="op", bufs=2, space="PSUM"))

    # Constants into SBUF, first-use order, each large tensor split into
    # two half-loads so completion semaphores fire as early as possible
    # (the warmup chain q_c0 -> ropeCS multiply -> q_c1 is gated on them).
    halves = {}

    def load(pool, ap, shape, tag):
        t = pool.tile(shape, BF16, tag=tag)
        halves[tag] = (t, ap)
        return t

    def load_half(tag, hf):
        t, ap = halves[tag]
        w = t.shape[-1] // 2
        sl = slice(hf * w, (hf + 1) * w)
        nc.sync.dma_start(t[:, sl], ap[:, sl])

    xT_sb = load(const, xT, [P, S], "xT")
    wqT_sb = load(const, wqT, [P, NH * P], "wqT")
    wqpT_sb = load(const, wqpT, [P, NH * P], "wqpT")
    ropeCS_sb = load(const, ropeCS, [P, 2 * S], "ropeCS")
    wkT_sb = load(const, wkT, [P, NH * P], "wkT")
    wkpT_sb = load(const, wkpT, [P, NH * P], "wkpT")
    wvT_sb = load(const, wvT, [P, NH * P], "wvT")
    woT_sb = load(const, woT, [P, NH * P], "woT")
    for tag in ("xT", "wqT", "wqpT", "ropeCS", "wvT", "wkT", "wkpT"):
        load_half(tag, 0)
    for tag in ("xT", "wqT", "wqpT", "ropeCS", "wvT", "wkT", "wkpT"):
        load_half(tag, 1)
    tri_sb = const.tile([P, P], BF16, tag="tri")
    nc.sync.dma_start(tri_sb[:], tri)
    idn_sb = const.tile([P, P], BF16, tag="idn")
    nc.sync.dma_start(idn_sb[:], idn)
    ones_sb = const.tile([P, P], BF16, tag="ones")
    nc.sync.dma_start(ones_sb[:], ones)
    load_half("woT", 0), load_half("woT", 1)

    # v for all heads, [s_in_tile, s_tile, head*128+d]
    v_sb = vpool.tile([P, NT, NH * P], BF16, tag="v")

    # Warm up the PE clock while the input DMAs stream: HAM only grants
    # full clock after ~3us of continuous busy, so a dozen dummy matmuls
    # on a memset tile let the first real matmuls run at 2.4GHz instead
    # of ramping through them.
    warm = opool.tile([P, 512], BF16, tag="warm")
    nc.gpsimd.memset(warm[:], 0.0)
    for i in range(16):
        wps = sp.tile([P, 512], F32, tag="att", name=f"warm{i}")
        nc.tensor.matmul(wps[:], warm[:, 0:P], warm[:], start=True, stop=True)

    def vproj_piece(st_i, c):
        """One (s-tile, head-half) piece of the V projection. Pieces are
        spread over the first three iterations in need-order: the c==0
        half (heads 0-3) lands during it=0, c==1 trickles in behind it,
        so the Scalar engine's copies never delay the first head's exps."""
        vp = sp.tile([P, 512], F32, tag="att", name=f"vp{st_i}_{c}")
        nc.tensor.matmul(vp[:], xT_sb[:, st_i * P:(st_i + 1) * P],
                         wvT_sb[:, c * 512:(c + 1) * 512],
                         start=True, stop=True)
        nc.scalar.copy(v_sb[:, st_i, c * 512:(c + 1) * 512], vp[:])

    # persistent output accumulator psum (2 banks)
    out_ps = [op.tile([P, 512], F32, tag="out", name=f"out_ps{c}")
              for c in range(2)]

    qrot = {}
    krot = {}
    ynTs = {}
    state = {}

    def proj_chunk(h, qk, c):
        """One 512-chunk of head h's q/qp (qk=0) or k/kp (qk=1) projection.
        Both matmuls land in one 2-bank PSUM pair so a single DVE multiply
        against the fused [C|S] table produces both RoPE products; the
        per-chunk add (bf16 SBUF, GpSimd) completes rot[:, chunk] early so
        the next head's S^T tiles are not gated on the full-row rotation."""
        if (h, qk) not in state:
            dst = qkpool.tile([P, S], BF16, tag=("qrot", "krot")[qk],
                              name=f"rot{h}_{qk}")
            state[(h, qk)] = dst
            (qrot, krot)[qk][h] = dst
        dst = state[(h, qk)]
        wt = (wqT_sb, wkT_sb)[qk][:, h * P:(h + 1) * P]
        wpt = (wqpT_sb, wkpT_sb)[qk][:, h * P:(h + 1) * P]
        sl = slice(c * 512, (c + 1) * 512)
        ab = pp.tile([P, 1024], F32, tag="proj", name=f"pab{h}_{qk}_{c}")
        nc.tensor.matmul(ab[:, 0:512], wt, xT_sb[:, sl], start=True, stop=True)
        nc.tensor.matmul(ab[:, 512:1024], wpt, xT_sb[:, sl],
                         start=True, stop=True)
        cs = tmppool.tile([P, 1024], BF16, tag=f"tmp{qk}",
                          name=f"cs{h}_{qk}_{c}")
        nc.vector.tensor_tensor(cs[:], ab[:], ropeCS_sb[:, c * 1024:
                                                        (c + 1) * 1024], MULT)
        eng = nc.vector if h == 0 else nc.gpsimd
        eng.tensor_tensor(dst[:, sl], cs[:, 0:512], cs[:, 512:1024], ADD)

    def st_tile(g, j, pT):
        """S^T block j for head g + exp."""
        qr, kr = qrot[g], krot[g]
        kblk = kr[:, j * P:(j + 1) * P]
        chunks = [(j * P, 512), (512, 1024)] if j < 4 else [(j * P, 1024)]
        # the last head runs with no projection work interleaved; borrow
        # the idle proj psum pair to deepen its S^T ring
        xtra = (g == NH - 1 and j >= 4 and j % 2 == 0)
        for ci, (a, bnd) in enumerate(chunks):
            w = bnd - a
            if xtra:
                stt = pp.tile([P, 512], F32, tag="proj", name=f"sx{g}_{j}")
            else:
                stt = sp.tile([P, 512], F32, tag="att")
            diag = (ci == 0)
            nc.tensor.matmul(stt[:, :w], kblk, qr[:, a:bnd],
                             start=True, stop=not diag)
            if diag:
                nc.tensor.matmul(stt[:, :P], idn_sb[:], tri_sb[:],
                                 start=False, stop=True)
            nc.scalar.activation(pT[:, j, a:bnd], stt[:, :w], Exp,
                                 scale=SCALE)

    def rs_chunk(g, c, pT, ri, jrange, ps):
        """Part of the rowsum accumulation for chunk c (ones-matmul gives
        the k-sum pre-broadcast across partitions)."""
        jmax = 4 * c + 3
        for j in jrange:
            r0 = max(c * 512, j * P)
            r1 = (c + 1) * 512
            nc.tensor.matmul(ps[:, r0 - c * 512:r1 - c * 512],
                             ones_sb[:], pT[:, j, r0:r1],
                             start=(j == 0), stop=(j == jmax))
        if jrange[-1] == jmax:
            nc.vector.reciprocal_approx_fast(ri[:, c * 512:(c + 1) * 512],
                                             ps[:, :512])

    def av_chunk(g, c, pT, ri, ynT, ps, emit_ynt=True):
        """AV accumulation + normalization for chunk c."""
        jmax = 4 * c + 3
        for j in range(jmax + 1):
            r0 = max(c * 512, j * P)
            r1 = (c + 1) * 512
            nc.tensor.matmul(ps[:, r0 - c * 512:r1 - c * 512],
                             v_sb[:, j, g * P:(g + 1) * P],
                             pT[:, j, r0:r1],
                             start=(j == 0), stop=(j == jmax))
        if emit_ynt:
            emit_ynt_mult(c, ri, ynT, ps)

    def emit_ynt_mult(c, ri, ynT, ps):
        nc.vector.tensor_tensor(ynT[:, c * 512:(c + 1) * 512],
                                ps[:, :512],
                                ri[:, c * 512:(c + 1) * 512], MULT)

    def emit_outproj(g, cs=(0, 1)):
        ynT = ynTs[g]
        for c in cs:
            nc.tensor.matmul(out_ps[c][:], woT_sb[:, g * P:(g + 1) * P],
                             ynT[:, c * 512:(c + 1) * 512],
                             start=(g == 0), stop=(g == NH - 1))

    # Software-pipelined head loop. Head h's projection matmuls and head
    # g=h-1's S^T / rowsum / AV matmuls are interleaved instruction by
    # instruction so the PE always has independent work queued while the
    # Scalar engine drains exps (S^T tiles are paced by the 3-buffer PSUM
    # ring) -- idle PE gaps trigger HAM clock throttling, which is worth
    # more than the gaps themselves. The rowsum/AV accumulation groups
    # rotate through one dedicated PSUM bank: rs_c0 -> av_c0 -> rs_c1 ->
    # av_c1, each WAR-dependency hidden behind interleaved S^T work.
    out_sb = opool.tile([P, S], F32, tag="osb")

    def finish_out(c):
        sl = slice(c * 512, (c + 1) * 512)
        nc.scalar.copy(out_sb[:, sl], out_ps[c][:])
        nc.sync.dma_start(outT[:, sl], out_sb[:, sl])

    res = {}
    st_done = {}

    def get_res(g):
        if g not in res:
            pT = ppool.tile([P, NT, S], BF16, tag="pT", name=f"pT{g}")
            ri = npool.tile([P, S], F32, tag="ri", name=f"ri{g}")
            ynT = npool.tile([P, S], BF16, tag="ynT", name=f"ynT{g}")
            res[g] = (pT, ri, ynT)
            st_done[g] = 0
        return res[g]

    def st_next(g, upto):
        pT = get_res(g)[0]
        while st_done[g] < upto:
            st_tile(g, st_done[g], pT)
            st_done[g] += 1

    deferred = {}
    for it in range(NH + 2):
        h = it if it < NH else None
        g = it - 1 if 1 <= it <= NH else None
        if g is not None:
            pT, ri, ynT = get_res(g)

        if g is not None:
            st_next(g, 1)
        davp = None
        if it - 2 in deferred:
            # previous head's AV_c1: its recip ran on DVE while this head's
            # first S^T tile kept the PE busy; the ynT multiply is emitted
            # after the q projection so it doesn't delay this head's RoPE
            # multiplies in the DVE stream
            dpT, dri, dynT = deferred.pop(it - 2)
            if it - 2 == NH - 1:
                # the proj psum pair is idle by now; using it lets the AV
                # matmuls overlap the rowsum/recip drain of the final head
                davp = pp.tile([P, 512], F32, tag="proj", name="av1_last")
            else:
                davp = ap_.tile([P, 512], F32, tag="avrs", name=f"av1_{it-2}")
            av_chunk(it - 2, 1, dpT, dri, dynT, davp, emit_ynt=False)
        vps = {0: [(0, 0), (1, 0), (2, 0), (3, 0),
                   (4, 0), (5, 0), (6, 0), (7, 0)],
               1: [(0, 1), (1, 1), (2, 1), (3, 1)],
               2: [(4, 1), (5, 1), (6, 1), (7, 1)]}.get(it, [])
        nvp = 2 if it == 0 else 1
        # at it=0 run the k chunk-0 projection right after q's so the
        # first S^T tile's DVE dependency chain completes sooner
        order = ((0, 0), (1, 0), (0, 1), (1, 1)) if it == 0 else \
                ((0, 0), (0, 1), (1, 0), (1, 1))

        def vp_fill():
            for st_i, c in vps[:nvp]:
                vproj_piece(st_i, c)
            del vps[:nvp]

        if h is not None:
            proj_chunk(h, *order[0])
            vp_fill()
        if davp is not None:
            emit_ynt_mult(1, dri, dynT, davp)
            ynTs[it - 2] = dynT
            if it - 2 == NH - 1:
                emit_outproj(NH - 1, cs=(1,))
                finish_out(1)  # last head: close out chunk 1 right away
        if g is not None:
            st_next(g, 2)
        if h is not None:
            proj_chunk(h, *order[1])
            vp_fill()
        if g is not None:
            st_next(g, 3)
        if h is not None:
            proj_chunk(h, *order[2])
            vp_fill()
        if g is not None:
            st_next(g, 4)
        if 2 <= it and it - 2 < NH - 1:
            emit_outproj(it - 2)  # deferred: ynT computed last iteration
        if h is not None:
            proj_chunk(h, *order[3])
            vp_fill()
        if h == NH - 1:
            # the final head's q/k rotations are ready mid-iteration; pull
            # its first S^T tiles forward so the drain iteration shrinks
            st_next(NH - 1, 3)
        if g is not None:
            st_tile(g, 4, pT)
            rs_ps = ap_.tile([P, 512], F32, tag="avrs", name=f"rs0_{g}")
            rs_chunk(g, 0, pT, ri, [0, 1, 2, 3], rs_ps)
            st_tile(g, 5, pT)
            av_ps = ap_.tile([P, 512], F32, tag="avrs", name=f"av0_{g}")
            av_chunk(g, 0, pT, ri, ynT, av_ps)
            if g == NH - 1:
                ynTs[g] = ynT
                emit_outproj(g, cs=(0,))
                finish_out(0)  # last head: close out chunk 0 early
            st_tile(g, 6, pT)
            rs_ps1 = ap_.tile([P, 512], F32, tag="avrs", name=f"rs1_{g}")
            rs_chunk(g, 1, pT, ri, [0, 1, 2, 3], rs_ps1)
            st_tile(g, 7, pT)
            rs_chunk(g, 1, pT, ri, [4, 5, 6, 7], rs_ps1)
            deferred[g] = (pT, ri, ynT)
            if g > 0:
                qrot.pop(g), krot.pop(g)
                state.pop((g, 0)), state.pop((g, 1))

    ctx.close()


def _rope_tables_np():
    """Bit-faithful replication of reference._rope_tables (float32 jax ops)."""
    import jax.numpy as jnp
    half = E // 2
    dtype = jnp.float32
    angles = jnp.power(jnp.asarray(10000.0, dtype),
                       2.0 * jnp.arange(half, dtype=dtype) / E)
    theta = jnp.arange(S, dtype=dtype)[:, None] * angles[None, :]
    return np.asarray(jnp.cos(theta)), np.asarray(jnp.sin(theta))


def make_in_maps(x, w_q, w_k, w_v, w_o):
    x = np.asarray(x, np.float32)
    w_q = np.asarray(w_q, np.float32)
    w_k = np.asarray(w_k, np.float32)
    w_v = np.asarray(w_v, np.float32)
    w_o = np.asarray(w_o, np.float32)

    def b16(a):
        return np.ascontiguousarray(a).astype(ml_dtypes.bfloat16)

    cos, sin = _rope_tables_np()            # [S, 64] f32
    ropeC = np.repeat(cos.T, 2, axis=0)     # [128, S]
    ropeS = np.repeat(sin.T, 2, axis=0)
    ropeS[0::2] *= -1.0
    # fused per-chunk [C | S] table so one DVE multiply covers both RoPE
    # products of a 512-column chunk
    ropeCS = np.concatenate([ropeC[:, 0:512], ropeS[:, 0:512],
                             ropeC[:, 512:1024], ropeS[:, 512:1024]], axis=1)

    tri = np.where(np.arange(P)[None, :] < np.arange(P)[:, None],
                   np.float32(-1e30), np.float32(0.0))
    idn = np.eye(P, dtype=np.float32)

    perm = np.arange(P)
    perm = perm ^ 1  # swap adjacent pairs

    def blocksT(w, heads, permute=False):
        # w: (2048, 128); heads: list of global head indices
        # -> (128, len*128) with column block j = w[h_j*128:(h_j+1)*128].T
        cols = []
        for hgl in heads:
            blk = w[hgl * P:(hgl + 1) * P, :]
            if permute:
                blk = blk[perm, :]
            cols.append(blk.T)
        return np.concatenate(cols, axis=1)

    in_maps = []
    for core in range(NCORES):
        b = core // 2
        g = core % 2
        heads = [g * NH + j for j in range(NH)]
        woTc = np.concatenate(
            [w_o[:, h * P:(h + 1) * P].T for h in heads], axis=1)
        in_maps.append({
            "xT": b16(x[b].T),
            "wqT": b16(blocksT(w_q, heads)),
            "wqpT": b16(blocksT(w_q, heads, permute=True)),
            "wkT": b16(blocksT(w_k, heads)),
            "wkpT": b16(blocksT(w_k, heads, permute=True)),
            "wvT": b16(blocksT(w_v, heads)),
            "woT": b16(woTc),
            "ropeCS": b16(ropeCS),
            "tri": b16(tri),
            "idn": b16(idn),
            "ones": np.ones((P, P), ml_dtypes.bfloat16),
        })
    return in_maps


_NC_CACHE = {}


def get_nc():
    if "nc" not in _NC_CACHE:
        _NC_CACHE["nc"] = build_bass()
    return _NC_CACHE["nc"]


def run(x, w_q, w_k, w_v, w_o, trace=False, trace_cores=None):
    nc = get_nc()
    in_maps = make_in_maps(x, w_q, w_k, w_v, w_o)
    res = run_bass_kernel_spmd(nc, in_maps, list(range(NCORES)), trace=trace,
                               trace_cores=trace_cores)
    out = np.zeros((B, S, E), np.float32)
    for core in range(NCORES):
        out[core // 2] += res.results[core]["outT"].T
    return out, res


def kernel(x, w_q, w_k, w_v, w_o):
    out, _ = run(x, w_q, w_k, w_v, w_o)
    return out


# revision 38
# speedup vs baseline: 1.0017x; 1.0017x over previous
"""Multi-head attention (16 heads, RoPE, causal) Trainium2 Bass kernel.

Sharding: 8 cores = 4-way data-parallel over batch x 2-way tensor-parallel
over heads (each core: 1 batch, 8 heads). Per-core partial outputs (over its
8 heads) are summed pairwise on the host (the w_o "all-reduce").

v2: all-bf16 datapath. All matmul operands are bf16 (fast weight load, no
f32r small-moving penalty, half the input DMA bytes). The softmax rowsum is
computed by pre-accumulating the 8 P^T k-tiles on the Vector engine (bf16
SBUF adds run 2 elem/cycle) into one [128, S] tile, then a single ones-
matmul per head reduces over partitions -- replacing 4608 PE matmul columns
per head with 1024. Elementwise work (RoPE multiplies/adds, normalization,
reciprocal, v copies) is statically balanced across Vector / GpSimd /
Scalar so no engine exceeds the PE's matmul stream.

Per-core algorithm (S=1024, E=128 = head dim, 8 local heads):
  - xT [e, s] bf16; per-head wT [e, d] blocks give qT/kT in [d, s] layout.
  - RoPE: rot(q)T = ropeC (.) qT + ropeS (.) (perm q)T with perm via
    pair-swapped weight copies (extra projection matmuls).
  - S^T[k, q] blocks per 128-wide k tile; causal diagonal gets a -1e30
    upper-triangular bias via one bf16 idn x tri matmul into the same PSUM
    accumulation group; Scalar applies exp(scale*x) writing P^T bf16.
  - rowsums: DVE pre-sum of P^T tiles + one ones-matmul; fast reciprocal.
  - y^T[d, q] = sum_j v_j @ P^T_j, normalized by recip rowsums, then
    out^T[e, s] += woT_h.T @ ynT_h accumulated in PSUM across heads.
"""

import os
import sys

import ml_dtypes
import numpy as np

for _p in ("/opt/trn_rl_repo",):
    if os.path.isdir(_p) and _p not in sys.path:
        sys.path.append(_p)

import concourse.bass as bass  # noqa: E402
import concourse.tile as tile  # noqa: E402
from concourse import bacc, mybir  # noqa: E402
from concourse.bass_utils import run_bass_kernel_spmd  # noqa: E402

F32 = mybir.dt.float32
BF16 = mybir.dt.bfloat16

B, S, E, H = 4, 1024, 128, 16
NCORES = 8
NH = 8          # heads per core
P = 128
NT = S // P     # 8 seq tiles
SCALE = 1.0 / float(np.sqrt(np.float32(E)))
Exp = mybir.ActivationFunctionType.Exp
MULT = mybir.AluOpType.mult
ADD = mybir.AluOpType.add


def build_bass():
    nc = bacc.Bacc("TRN2", target_bir_lowering=False, debug=False,
                   num_devices=NCORES)

    def din(name, shape, dt=BF16):
        return nc.dram_tensor(name, shape, dt, kind="ExternalInput").ap()

    xT = din("xT", [P, S])
    wqT = din("wqT", [P, NH * P])
    wqpT = din("wqpT", [P, NH * P])
    wkT = din("wkT", [P, NH * P])
    wkpT = din("wkpT", [P, NH * P])
    wvT = din("wvT", [P, NH * P])
    woT = din("woT", [P, NH * P])
    ropeCS = din("ropeCS", [P, 2 * S])
    tri = din("tri", [P, P])
    idn = din("idn", [P, P])
    ones = din("ones", [P, P])
    outT = nc.dram_tensor("outT", [P, S], F32, kind="ExternalOutput").ap()

    with tile.TileContext(nc) as tc:
        _build(tc, xT, wqT, wqpT, wkT, wkpT, wvT, woT, ropeCS, tri,
               idn, ones, outT)
    nc.compile()
    return nc


def _build(tc, xT, wqT, wqpT, wkT, wkpT, wvT, woT, ropeCS, tri, idn,
           ones, outT):
    nc = tc.nc

    from contextlib import ExitStack
    ctx = ExitStack()
    const = ctx.enter_context(tc.tile_pool(name="const", bufs=1))
    vpool = ctx.enter_context(tc.tile_pool(name="vpool", bufs=1))
    ppool = ctx.enter_context(tc.tile_pool(name="ppool", bufs=2))
    qkpool = ctx.enter_context(tc.tile_pool(name="qkpool", bufs=2))
    tmppool = ctx.enter_context(tc.tile_pool(name="tmppool", bufs=2))
    npool = ctx.enter_context(tc.tile_pool(name="npool", bufs=2))
    opool = ctx.enter_context(tc.tile_pool(name="opool", bufs=1))
    # PSUM budget (8 banks): proj 2-bank pair, S^T ring 3, one rotating
    # bank for the rowsum/AV accumulation groups, outproj accumulators 2.
    pp = ctx.enter_context(tc.tile_pool(name="pp", bufs=1, space="PSUM"))
    sp = ctx.enter_context(tc.tile_pool(name="sp", bufs=3, space="PSUM"))
    ap_ = ctx.enter_context(tc.tile_pool(name="ap", bufs=1, space="PSUM"))
    op = ctx.enter_context(tc.tile_pool(name="op", bufs=2, space="PSUM"))

    # Constants into SBUF, first-use order, each large tensor split into
    # two half-loads so completion semaphores fire as early as possible
    # (the warmup chain q_c0 -> ropeCS multiply -> q_c1 is gated on them).
    halves = {}

    def load(pool, ap, shape, tag):
        t = pool.tile(shape, BF16, tag=tag)
        halves[tag] = (t, ap)
        return t

    def load_half(tag, hf):
        t, ap = halves[tag]
        w = t.shape[-1] // 2
        sl = slice(hf * w, (hf + 1) * w)
        nc.sync.dma_start(t[:, sl], ap[:, sl])

    xT_sb = load(const, xT, [P, S], "xT")
    wqT_sb = load(const, wqT, [P, NH * P], "wqT")
    wqpT_sb = load(const, wqpT, [P, NH * P], "wqpT")
    ropeCS_sb = load(const, ropeCS, [P, 2 * S], "ropeCS")
    wkT_sb = load(const, wkT, [P, NH * P], "wkT")
    wkpT_sb = load(const, wkpT, [P, NH * P], "wkpT")
    wvT_sb = load(const, wvT, [P, NH * P], "wvT")
    woT_sb = load(const, woT, [P, NH * P], "woT")
    for tag in ("xT", "wqT", "wqpT", "ropeCS", "wvT", "wkT", "wkpT"):
        load_half(tag, 0)
    for tag in ("xT", "wqT", "wqpT", "ropeCS", "wvT", "wkT", "wkpT"):
        load_half(tag, 1)
    tri_sb = const.tile([P, P], BF16, tag="tri")
    nc.sync.dma_start(tri_sb[:], tri)
    idn_sb = const.tile([P, P], BF16, tag="idn")
    nc.sync.dma_start(idn_sb[:], idn)
    ones_sb = const.tile([P, P], BF16, tag="ones")
    nc.sync.dma_start(ones_sb[:], ones)
    load_half("woT", 0), load_half("woT", 1)

    # v for all heads, [s_in_tile, s_tile, head*128+d]
    v_sb = vpool.tile([P, NT, NH * P], BF16, tag="v")

    # Warm up the PE clock while the input DMAs stream: HAM only grants
    # full clock after ~3us of continuous busy, so a dozen dummy matmuls
    # on a memset tile let the first real matmuls run at 2.4GHz instead
    # of ramping through them.
    warm = opool.tile([P, 512], BF16, tag="warm")
    nc.gpsimd.memset(warm[:], 0.0)
    for i in range(16):
        wps = sp.tile([P, 512], F32, tag="att", name=f"warm{i}")
        nc.tensor.matmul(wps[:], warm[:, 0:P], warm[:], start=True, stop=True)

    def vproj_piece(st_i, c):
        """One (s-tile, head-half) piece of the V projection. Pieces are
        spread over the first three iterations in need-order: the c==0
        half (heads 0-3) lands during it=0, c==1 trickles in behind it,
        so the Scalar engine's copies never delay the first head's exps."""
        vp = sp.tile([P, 512], F32, tag="att", name=f"vp{st_i}_{c}")
        nc.tensor.matmul(vp[:], xT_sb[:, st_i * P:(st_i + 1) * P],
                         wvT_sb[:, c * 512:(c + 1) * 512],
                         start=True, stop=True)
        nc.scalar.copy(v_sb[:, st_i, c * 512:(c + 1) * 512], vp[:])

    # persistent output accumulator psum (2 banks)
    out_ps = [op.tile([P, 512], F32, tag="out", name=f"out_ps{c}")
              for c in range(2)]

    qrot = {}
    krot = {}
    ynTs = {}
    state = {}

    def proj_chunk(h, qk, c):
        """One 512-chunk of head h's q/qp (qk=0) or k/kp (qk=1) projection.
        Both matmuls land in one 2-bank PSUM pair so a single DVE multiply
        against the fused [C|S] table produces both RoPE products; the
        per-chunk add (bf16 SBUF, GpSimd) completes rot[:, chunk] early so
        the next head's S^T tiles are not gated on the full-row rotation."""
        if (h, qk) not in state:
            dst = qkpool.tile([P, S], BF16, tag=("qrot", "krot")[qk],
                              name=f"rot{h}_{qk}")
            state[(h, qk)] = dst
            (qrot, krot)[qk][h] = dst
        dst = state[(h, qk)]
        wt = (wqT_sb, wkT_sb)[qk][:, h * P:(h + 1) * P]
        wpt = (wqpT_sb, wkpT_sb)[qk][:, h * P:(h + 1) * P]
        sl = slice(c * 512, (c + 1) * 512)
        ab = pp.tile([P, 1024], F32, tag="proj", name=f"pab{h}_{qk}_{c}")
        nc.tensor.matmul(ab[:, 0:512], wt, xT_sb[:, sl], start=True, stop=True)
        nc.tensor.matmul(ab[:, 512:1024], wpt, xT_sb[:, sl],
                         start=True, stop=True)
        cs = tmppool.tile([P, 1024], BF16, tag=f"tmp{qk}",
                          name=f"cs{h}_{qk}_{c}")
        nc.vector.tensor_tensor(cs[:], ab[:], ropeCS_sb[:, c * 1024:
                                                        (c + 1) * 1024], MULT)
        eng = nc.vector if h == 0 else nc.gpsimd
        eng.tensor_tensor(dst[:, sl], cs[:, 0:512], cs[:, 512:1024], ADD)

    def st_tile(g, j, pT):
        """S^T block j for head g + exp."""
        qr, kr = qrot[g], krot[g]
        kblk = kr[:, j * P:(j + 1) * P]
        chunks = [(j * P, 512), (512, 1024)] if j < 4 else [(j * P, 1024)]
        # the last head runs with no projection work interleaved; borrow
        # the idle proj psum pair to deepen its S^T ring
        xtra = (g == NH - 1 and j >= 4 and j % 2 == 0)
        for ci, (a, bnd) in enumerate(chunks):
            w = bnd - a
            if xtra:
                stt = pp.tile([P, 512], F32, tag="proj", name=f"sx{g}_{j}")
            else:
                stt = sp.tile([P, 512], F32, tag="att")
            diag = (ci == 0)
            nc.tensor.matmul(stt[:, :w], kblk, qr[:, a:bnd],
                             start=True, stop=not diag)
            if diag:
                nc.tensor.matmul(stt[:, :P], idn_sb[:], tri_sb[:],
                                 start=False, stop=True)
            nc.scalar.activation(pT[:, j, a:bnd], stt[:, :w], Exp,
                                 scale=SCALE)

    def rs_chunk(g, c, pT, ri, jrange, ps):
        """Part of the rowsum accumulation for chunk c (ones-matmul gives
        the k-sum pre-broadcast across partitions)."""
        jmax = 4 * c + 3
        for j in jrange:
            r0 = max(c * 512, j * P)
            r1 = (c + 1) * 512
            nc.tensor.matmul(ps[:, r0 - c * 512:r1 - c * 512],
                             ones_sb[:], pT[:, j, r0:r1],
                             start=(j == 0), stop=(j == jmax))
        if jrange[-1] == jmax:
            nc.vector.reciprocal_approx_fast(ri[:, c * 512:(c + 1) * 512],
                                             ps[:, :512])

    def av_chunk(g, c, pT, ri, ynT, ps, emit_ynt=True):
        """AV accumulation + normalization for chunk c."""
        jmax = 4 * c + 3
        for j in range(jmax + 1):
            r0 = max(c * 512, j * P)
            r1 = (c + 1) * 512
            nc.tensor.matmul(ps[:, r0 - c * 512:r1 - c * 512],
                             v_sb[:, j, g * P:(g + 1) * P],
                             pT[:, j, r0:r1],
                             start=(j == 0), stop=(j == jmax))
        if emit_ynt:
            emit_ynt_mult(c, ri, ynT, ps)

    def emit_ynt_mult(c, ri, ynT, ps):
        nc.vector.tensor_tensor(ynT[:, c * 512:(c + 1) * 512],
                                ps[:, :512],
                                ri[:, c * 512:(c + 1) * 512], MULT)

    def emit_outproj(g, cs=(0, 1)):
        ynT = ynTs[g]
        for c in cs:
            nc.tensor.matmul(out_ps[c][:], woT_sb[:, g * P:(g + 1) * P],
                             ynT[:, c * 512:(c + 1) * 512],
                             start=(g == 0), stop=(g == NH - 1))

    # Software-pipelined head loop. Head h's projection matmuls and head
    # g=h-1's S^T / rowsum / AV matmuls are interleaved instruction by
    # instruction so the PE always has independent work queued while the
    # Scalar engine drains exps (S^T tiles are paced by the 3-buffer PSUM
    # ring) -- idle PE gaps trigger HAM clock throttling, which is worth
    # more than the gaps themselves. The rowsum/AV accumulation groups
    # rotate through one dedicated PSUM bank: rs_c0 -> av_c0 -> rs_c1 ->
    # av_c1, each WAR-dependency hidden behind interleaved S^T work.
    out_sb = opool.tile([P, S], F32, tag="osb")

    def finish_out(c):
        sl = slice(c * 512, (c + 1) * 512)
        nc.scalar.copy(out_sb[:, sl], out_ps[c][:])
        nc.sync.dma_start(outT[:, sl], out_sb[:, sl])

    res = {}
    st_done = {}

    def get_res(g):
        if g not in res:
            pT = ppool.tile([P, NT, S], BF16, tag="pT", name=f"pT{g}")
            ri = npool.tile([P, S], F32, tag="ri", name=f"ri{g}")
            ynT = npool.tile([P, S], BF16, tag="ynT", name=f"ynT{g}")
            res[g] = (pT, ri, ynT)
            st_done[g] = 0
        return res[g]

    def st_next(g, upto):
        pT = get_res(g)[0]
        while st_done[g] < upto:
            st_tile(g, st_done[g], pT)
            st_done[g] += 1

    deferred = {}
    for it in range(NH + 2):
        h = it if it < NH else None
        g = it - 1 if 1 <= it <= NH else None
        if g is not None:
            pT, ri, ynT = get_res(g)

        if g is not None:
            st_next(g, 1)
        davp = None
        if it - 2 in deferred:
            # previous head's AV_c1: its recip ran on DVE while this head's
            # first S^T tile kept the PE busy; the ynT multiply is emitted
            # after the q projection so it doesn't delay this head's RoPE
            # multiplies in the DVE stream
            dpT, dri, dynT = deferred.pop(it - 2)
            if it - 2 == NH - 1:
                # the proj psum pair is idle by now; using it lets the AV
                # matmuls overlap the rowsum/recip drain of the final head
                davp = pp.tile([P, 512], F32, tag="proj", name="av1_last")
            else:
                davp = ap_.tile([P, 512], F32, tag="avrs", name=f"av1_{it-2}")
            av_chunk(it - 2, 1, dpT, dri, dynT, davp, emit_ynt=False)
        vps = {0: [(0, 0), (1, 0), (2, 0), (3, 0),
                   (4, 0), (5, 0), (6, 0), (7, 0)],
               1: [(0, 1), (1, 1), (2, 1), (3, 1)],
               2: [(4, 1), (5, 1), (6, 1), (7, 1)]}.get(it, [])
        nvp = 2 if it == 0 else 1
        # at it=0 run the k chunk-0 projection right after q's so the
        # first S^T tile's DVE dependency chain completes sooner
        order = ((0, 0), (1, 0), (0, 1), (1, 1)) if it == 0 else \
                ((0, 0), (0, 1), (1, 0), (1, 1))

        def vp_fill():
            for st_i, c in vps[:nvp]:
                vproj_piece(st_i, c)
            del vps[:nvp]

        if h is not None:
            proj_chunk(h, *order[0])
            vp_fill()
        if davp is not None:
            emit_ynt_mult(1, dri, dynT, davp)
            ynTs[it - 2] = dynT
            if it - 2 == NH - 1:
                emit_outproj(NH - 1, cs=(1,))
                finish_out(1)  # last head: close out chunk 1 right away
        if g is not None:
            st_next(g, 2)
        if h is not None:
            proj_chunk(h, *order[1])
            vp_fill()
        if g is not None:
            st_next(g, 3)
        if h is not None:
            proj_chunk(h, *order[2])
            vp_fill()
        if g is not None:
            st_next(g, 4)
        if 2 <= it and it - 2 < NH - 1:
            emit_outproj(it - 2)  # deferred: ynT computed last iteration
        if h is not None:
            proj_chunk(h, *order[3])
            vp_fill()
        if g is not None:
            st_tile(g, 4, pT)
            rs_ps = ap_.tile([P, 512], F32, tag="avrs", name=f"rs0_{g}")
            rs_chunk(g, 0, pT, ri, [0, 1, 2, 3], rs_ps)
            st_tile(g, 5, pT)
            av_ps = ap_.tile([P, 512], F32, tag="avrs", name=f"av0_{g}")
            av_chunk(g, 0, pT, ri, ynT, av_ps)
            if g == NH - 1:
                ynTs[g] = ynT
                emit_outproj(g, cs=(0,))
                finish_out(0)  # last head: close out chunk 0 early
            st_tile(g, 6, pT)
            rs_ps1 = ap_.tile([P, 512], F32, tag="avrs", name=f"rs1_{g}")
            rs_chunk(g, 1, pT, ri, [0, 1, 2, 3], rs_ps1)
            st_tile(g, 7, pT)
            rs_chunk(g, 1, pT, ri, [4, 5, 6, 7], rs_ps1)
            deferred[g] = (pT, ri, ynT)
            if g > 0:
                qrot.pop(g), krot.pop(g)
                state.pop((g, 0)), state.pop((g, 1))

    ctx.close()


def _rope_tables_np():
    """Bit-faithful replication of reference._rope_tables (float32 jax ops)."""
    import jax.numpy as jnp
    half = E // 2
    dtype = jnp.float32
    angles = jnp.power(jnp.asarray(10000.0, dtype),
                       2.0 * jnp.arange(half, dtype=dtype) / E)
    theta = jnp.arange(S, dtype=dtype)[:, None] * angles[None, :]
    return np.asarray(jnp.cos(theta)), np.asarray(jnp.sin(theta))


def make_in_maps(x, w_q, w_k, w_v, w_o):
    x = np.asarray(x, np.float32)
    w_q = np.asarray(w_q, np.float32)
    w_k = np.asarray(w_k, np.float32)
    w_v = np.asarray(w_v, np.float32)
    w_o = np.asarray(w_o, np.float32)

    def b16(a):
        return np.ascontiguousarray(a).astype(ml_dtypes.bfloat16)

    cos, sin = _rope_tables_np()            # [S, 64] f32
    ropeC = np.repeat(cos.T, 2, axis=0)     # [128, S]
    ropeS = np.repeat(sin.T, 2, axis=0)
    ropeS[0::2] *= -1.0
    # fused per-chunk [C | S] table so one DVE multiply covers both RoPE
    # products of a 512-column chunk
    ropeCS = np.concatenate([ropeC[:, 0:512], ropeS[:, 0:512],
                             ropeC[:, 512:1024], ropeS[:, 512:1024]], axis=1)

    tri = np.where(np.arange(P)[None, :] < np.arange(P)[:, None],
                   np.float32(-1e30), np.float32(0.0))
    idn = np.eye(P, dtype=np.float32)

    perm = np.arange(P)
    perm = perm ^ 1  # swap adjacent pairs

    def blocksT(w, heads, permute=False):
        # w: (2048, 128); heads: list of global head indices
        # -> (128, len*128) with column block j = w[h_j*128:(h_j+1)*128].T
        cols = []
        for hgl in heads:
            blk = w[hgl * P:(hgl + 1) * P, :]
            if permute:
                blk = blk[perm, :]
            cols.append(blk.T)
        return np.concatenate(cols, axis=1)

    in_maps = []
    for core in range(NCORES):
        b = core // 2
        g = core % 2
        heads = [g * NH + j for j in range(NH)]
        woTc = np.concatenate(
            [w_o[:, h * P:(h + 1) * P].T for h in heads], axis=1)
        in_maps.append({
            "xT": b16(x[b].T),
            "wqT": b16(blocksT(w_q, heads)),
            "wqpT": b16(blocksT(w_q, heads, permute=True)),
            "wkT": b16(blocksT(w_k, heads)),
            "wkpT": b16(blocksT(w_k, heads, permute=True)),
            "wvT": b16(blocksT(w_v, heads)),
            "woT": b16(woTc),
            "ropeCS": b16(ropeCS),
            "tri": b16(tri),
            "idn": b16(idn),
            "ones": np.ones((P, P), ml_dtypes.bfloat16),
        })
    return in_maps


_NC_CACHE = {}


def get_nc():
    if "nc" not in _NC_CACHE:
        _NC_CACHE["nc"] = build_bass()
    return _NC_CACHE["nc"]


def run(x, w_q, w_k, w_v, w_o, trace=False, trace_cores=None):
    nc = get_nc()
    in_maps = make_in_maps(x, w_q, w_k, w_v, w_o)
    res = run_bass_kernel_spmd(nc, in_maps, list(range(NCORES)), trace=trace,
                               trace_cores=trace_cores)
    out = np.zeros((B, S, E), np.float32)
    for core in range(NCORES):
        out[core // 2] += res.results[core]["outT"].T
    return out, res


def kernel(x, w_q, w_k, w_v, w_o):
    out, _ = run(x, w_q, w_k, w_v, w_o)
    return out


# revision 39
# speedup vs baseline: 1.0028x; 1.0011x over previous
"""Multi-head attention (16 heads, RoPE, causal) Trainium2 Bass kernel.

Sharding: 8 cores = 4-way data-parallel over batch x 2-way tensor-parallel
over heads (each core: 1 batch, 8 heads). Per-core partial outputs (over its
8 heads) are summed pairwise on the host (the w_o "all-reduce").

v2: all-bf16 datapath. All matmul operands are bf16 (fast weight load, no
f32r small-moving penalty, half the input DMA bytes). The softmax rowsum is
computed by pre-accumulating the 8 P^T k-tiles on the Vector engine (bf16
SBUF adds run 2 elem/cycle) into one [128, S] tile, then a single ones-
matmul per head reduces over partitions -- replacing 4608 PE matmul columns
per head with 1024. Elementwise work (RoPE multiplies/adds, normalization,
reciprocal, v copies) is statically balanced across Vector / GpSimd /
Scalar so no engine exceeds the PE's matmul stream.

Per-core algorithm (S=1024, E=128 = head dim, 8 local heads):
  - xT [e, s] bf16; per-head wT [e, d] blocks give qT/kT in [d, s] layout.
  - RoPE: rot(q)T = ropeC (.) qT + ropeS (.) (perm q)T with perm via
    pair-swapped weight copies (extra projection matmuls).
  - S^T[k, q] blocks per 128-wide k tile; causal diagonal gets a -1e30
    upper-triangular bias via one bf16 idn x tri matmul into the same PSUM
    accumulation group; Scalar applies exp(scale*x) writing P^T bf16.
  - rowsums: DVE pre-sum of P^T tiles + one ones-matmul; fast reciprocal.
  - y^T[d, q] = sum_j v_j @ P^T_j, normalized by recip rowsums, then
    out^T[e, s] += woT_h.T @ ynT_h accumulated in PSUM across heads.
"""

import os
import sys

import ml_dtypes
import numpy as np

for _p in ("/opt/trn_rl_repo",):
    if os.path.isdir(_p) and _p not in sys.path:
        sys.path.append(_p)

import concourse.bass as bass  # noqa: E402
import concourse.tile as tile  # noqa: E402
from concourse import bacc, mybir  # noqa: E402
from concourse.bass_utils import run_bass_kernel_spmd  # noqa: E402

F32 = mybir.dt.float32
BF16 = mybir.dt.bfloat16

B, S, E, H = 4, 1024, 128, 16
NCORES = 8
NH = 8          # heads per core
P = 128
NT = S // P     # 8 seq tiles
SCALE = 1.0 / float(np.sqrt(np.float32(E)))
Exp = mybir.ActivationFunctionType.Exp
MULT = mybir.AluOpType.mult
ADD = mybir.AluOpType.add


def build_bass():
    nc = bacc.Bacc("TRN2", target_bir_lowering=False, debug=False,
                   num_devices=NCORES)

    def din(name, shape, dt=BF16):
        return nc.dram_tensor(name, shape, dt, kind="ExternalInput").ap()

    xT = din("xT", [P, S])
    wqT = din("wqT", [P, NH * P])
    wqpT = din("wqpT", [P, NH * P])
    wkT = din("wkT", [P, NH * P])
    wkpT = din("wkpT", [P, NH * P])
    wvT = din("wvT", [P, NH * P])
    woT = din("woT", [P, NH * P])
    ropeCS = din("ropeCS", [P, 2 * S])
    tri = din("tri", [P, P])
    idn = din("idn", [P, P])
    ones = din("ones", [P, P])
    outT = nc.dram_tensor("outT", [P, S], F32, kind="ExternalOutput").ap()

    with tile.TileContext(nc) as tc:
        _build(tc, xT, wqT, wqpT, wkT, wkpT, wvT, woT, ropeCS, tri,
               idn, ones, outT)
    nc.compile()
    return nc


def _build(tc, xT, wqT, wqpT, wkT, wkpT, wvT, woT, ropeCS, tri, idn,
           ones, outT):
    nc = tc.nc

    from contextlib import ExitStack
    ctx = ExitStack()
    const = ctx.enter_context(tc.tile_pool(name="const", bufs=1))
    vpool = ctx.enter_context(tc.tile_pool(name="vpool", bufs=1))
    ppool = ctx.enter_context(tc.tile_pool(name="ppool", bufs=2))
    qkpool = ctx.enter_context(tc.tile_pool(name="qkpool", bufs=2))
    tmppool = ctx.enter_context(tc.tile_pool(name="tmppool", bufs=2))
    npool = ctx.enter_context(tc.tile_pool(name="npool", bufs=2))
    opool = ctx.enter_context(tc.tile_pool(name="opool", bufs=1))
    # PSUM budget (8 banks): proj 2-bank pair, S^T ring 3, one rotating
    # bank for the rowsum/AV accumulation groups, outproj accumulators 2.
    pp = ctx.enter_context(tc.tile_pool(name="pp", bufs=1, space="PSUM"))
    sp = ctx.enter_context(tc.tile_pool(name="sp", bufs=3, space="PSUM"))
    ap_ = ctx.enter_context(tc.tile_pool(name="ap", bufs=1, space="PSUM"))
    op = ctx.enter_context(tc.tile_pool(name="op", bufs=2, space="PSUM"))

    # Constants into SBUF, first-use order, each large tensor split into
    # two half-loads so completion semaphores fire as early as possible
    # (the warmup chain q_c0 -> ropeCS multiply -> q_c1 is gated on them).
    halves = {}

    def load(pool, ap, shape, tag):
        t = pool.tile(shape, BF16, tag=tag)
        halves[tag] = (t, ap)
        return t

    def load_half(tag, hf):
        t, ap = halves[tag]
        w = t.shape[-1] // 2
        sl = slice(hf * w, (hf + 1) * w)
        nc.sync.dma_start(t[:, sl], ap[:, sl])

    xT_sb = load(const, xT, [P, S], "xT")
    wqT_sb = load(const, wqT, [P, NH * P], "wqT")
    wqpT_sb = load(const, wqpT, [P, NH * P], "wqpT")
    ropeCS_sb = load(const, ropeCS, [P, 2 * S], "ropeCS")
    wkT_sb = load(const, wkT, [P, NH * P], "wkT")
    wkpT_sb = load(const, wkpT, [P, NH * P], "wkpT")
    wvT_sb = load(const, wvT, [P, NH * P], "wvT")
    woT_sb = load(const, woT, [P, NH * P], "woT")
    for tag in ("xT", "wqT", "wqpT", "ropeCS", "wvT", "wkT", "wkpT"):
        load_half(tag, 0)
    for tag in ("xT", "wqT", "wqpT", "ropeCS", "wvT", "wkT", "wkpT"):
        load_half(tag, 1)
    tri_sb = const.tile([P, P], BF16, tag="tri")
    nc.sync.dma_start(tri_sb[:], tri)
    idn_sb = const.tile([P, P], BF16, tag="idn")
    nc.sync.dma_start(idn_sb[:], idn)
    ones_sb = const.tile([P, P], BF16, tag="ones")
    nc.sync.dma_start(ones_sb[:], ones)
    load_half("woT", 0), load_half("woT", 1)

    # v for all heads, [s_in_tile, s_tile, head*128+d]
    v_sb = vpool.tile([P, NT, NH * P], BF16, tag="v")

    # Warm up the PE clock while the input DMAs stream: HAM only grants
    # full clock after ~3us of continuous busy, so a dozen dummy matmuls
    # on a memset tile let the first real matmuls run at 2.4GHz instead
    # of ramping through them.
    warm = opool.tile([P, 512], BF16, tag="warm")
    nc.gpsimd.memset(warm[:], 0.0)
    for i in range(22):
        wps = sp.tile([P, 512], F32, tag="att", name=f"warm{i}")
        nc.tensor.matmul(wps[:], warm[:, 0:P], warm[:], start=True, stop=True)

    def vproj_piece(st_i, c):
        """One (s-tile, head-half) piece of the V projection. Pieces are
        spread over the first three iterations in need-order: the c==0
        half (heads 0-3) lands during it=0, c==1 trickles in behind it,
        so the Scalar engine's copies never delay the first head's exps."""
        vp = sp.tile([P, 512], F32, tag="att", name=f"vp{st_i}_{c}")
        nc.tensor.matmul(vp[:], xT_sb[:, st_i * P:(st_i + 1) * P],
                         wvT_sb[:, c * 512:(c + 1) * 512],
                         start=True, stop=True)
        nc.scalar.copy(v_sb[:, st_i, c * 512:(c + 1) * 512], vp[:])

    # persistent output accumulator psum (2 banks)
    out_ps = [op.tile([P, 512], F32, tag="out", name=f"out_ps{c}")
              for c in range(2)]

    qrot = {}
    krot = {}
    ynTs = {}
    state = {}

    def proj_chunk(h, qk, c):
        """One 512-chunk of head h's q/qp (qk=0) or k/kp (qk=1) projection.
        Both matmuls land in one 2-bank PSUM pair so a single DVE multiply
        against the fused [C|S] table produces both RoPE products; the
        per-chunk add (bf16 SBUF, GpSimd) completes rot[:, chunk] early so
        the next head's S^T tiles are not gated on the full-row rotation."""
        if (h, qk) not in state:
            dst = qkpool.tile([P, S], BF16, tag=("qrot", "krot")[qk],
                              name=f"rot{h}_{qk}")
            state[(h, qk)] = dst
            (qrot, krot)[qk][h] = dst
        dst = state[(h, qk)]
        wt = (wqT_sb, wkT_sb)[qk][:, h * P:(h + 1) * P]
        wpt = (wqpT_sb, wkpT_sb)[qk][:, h * P:(h + 1) * P]
        sl = slice(c * 512, (c + 1) * 512)
        ab = pp.tile([P, 1024], F32, tag="proj", name=f"pab{h}_{qk}_{c}")
        nc.tensor.matmul(ab[:, 0:512], wt, xT_sb[:, sl], start=True, stop=True)
        nc.tensor.matmul(ab[:, 512:1024], wpt, xT_sb[:, sl],
                         start=True, stop=True)
        cs = tmppool.tile([P, 1024], BF16, tag=f"tmp{qk}",
                          name=f"cs{h}_{qk}_{c}")
        nc.vector.tensor_tensor(cs[:], ab[:], ropeCS_sb[:, c * 1024:
                                                        (c + 1) * 1024], MULT)
        eng = nc.vector if h == 0 else nc.gpsimd
        eng.tensor_tensor(dst[:, sl], cs[:, 0:512], cs[:, 512:1024], ADD)

    def st_tile(g, j, pT):
        """S^T block j for head g + exp."""
        qr, kr = qrot[g], krot[g]
        kblk = kr[:, j * P:(j + 1) * P]
        chunks = [(j * P, 512), (512, 1024)] if j < 4 else [(j * P, 1024)]
        # the last head runs with no projection work interleaved; borrow
        # the idle proj psum pair to deepen its S^T ring
        xtra = (g == NH - 1 and j >= 4 and j % 2 == 0)
        for ci, (a, bnd) in enumerate(chunks):
            w = bnd - a
            if xtra:
                stt = pp.tile([P, 512], F32, tag="proj", name=f"sx{g}_{j}")
            else:
                stt = sp.tile([P, 512], F32, tag="att")
            diag = (ci == 0)
            nc.tensor.matmul(stt[:, :w], kblk, qr[:, a:bnd],
                             start=True, stop=not diag)
            if diag:
                nc.tensor.matmul(stt[:, :P], idn_sb[:], tri_sb[:],
                                 start=False, stop=True)
            nc.scalar.activation(pT[:, j, a:bnd], stt[:, :w], Exp,
                                 scale=SCALE)

    def rs_chunk(g, c, pT, ri, jrange, ps):
        """Part of the rowsum accumulation for chunk c (ones-matmul gives
        the k-sum pre-broadcast across partitions)."""
        jmax = 4 * c + 3
        for j in jrange:
            r0 = max(c * 512, j * P)
            r1 = (c + 1) * 512
            nc.tensor.matmul(ps[:, r0 - c * 512:r1 - c * 512],
                             ones_sb[:], pT[:, j, r0:r1],
                             start=(j == 0), stop=(j == jmax))
        if jrange[-1] == jmax:
            nc.vector.reciprocal_approx_fast(ri[:, c * 512:(c + 1) * 512],
                                             ps[:, :512])

    def av_chunk(g, c, pT, ri, ynT, ps, emit_ynt=True):
        """AV accumulation + normalization for chunk c."""
        jmax = 4 * c + 3
        for j in range(jmax + 1):
            r0 = max(c * 512, j * P)
            r1 = (c + 1) * 512
            nc.tensor.matmul(ps[:, r0 - c * 512:r1 - c * 512],
                             v_sb[:, j, g * P:(g + 1) * P],
                             pT[:, j, r0:r1],
                             start=(j == 0), stop=(j == jmax))
        if emit_ynt:
            emit_ynt_mult(c, ri, ynT, ps)

    def emit_ynt_mult(c, ri, ynT, ps):
        nc.vector.tensor_tensor(ynT[:, c * 512:(c + 1) * 512],
                                ps[:, :512],
                                ri[:, c * 512:(c + 1) * 512], MULT)

    def emit_outproj(g, cs=(0, 1)):
        ynT = ynTs[g]
        for c in cs:
            nc.tensor.matmul(out_ps[c][:], woT_sb[:, g * P:(g + 1) * P],
                             ynT[:, c * 512:(c + 1) * 512],
                             start=(g == 0), stop=(g == NH - 1))

    # Software-pipelined head loop. Head h's projection matmuls and head
    # g=h-1's S^T / rowsum / AV matmuls are interleaved instruction by
    # instruction so the PE always has independent work queued while the
    # Scalar engine drains exps (S^T tiles are paced by the 3-buffer PSUM
    # ring) -- idle PE gaps trigger HAM clock throttling, which is worth
    # more than the gaps themselves. The rowsum/AV accumulation groups
    # rotate through one dedicated PSUM bank: rs_c0 -> av_c0 -> rs_c1 ->
    # av_c1, each WAR-dependency hidden behind interleaved S^T work.
    out_sb = opool.tile([P, S], F32, tag="osb")

    def finish_out(c):
        sl = slice(c * 512, (c + 1) * 512)
        nc.scalar.copy(out_sb[:, sl], out_ps[c][:])
        nc.sync.dma_start(outT[:, sl], out_sb[:, sl])

    res = {}
    st_done = {}

    def get_res(g):
        if g not in res:
            pT = ppool.tile([P, NT, S], BF16, tag="pT", name=f"pT{g}")
            ri = npool.tile([P, S], F32, tag="ri", name=f"ri{g}")
            ynT = npool.tile([P, S], BF16, tag="ynT", name=f"ynT{g}")
            res[g] = (pT, ri, ynT)
            st_done[g] = 0
        return res[g]

    def st_next(g, upto):
        pT = get_res(g)[0]
        while st_done[g] < upto:
            st_tile(g, st_done[g], pT)
            st_done[g] += 1

    deferred = {}
    for it in range(NH + 2):
        h = it if it < NH else None
        g = it - 1 if 1 <= it <= NH else None
        if g is not None:
            pT, ri, ynT = get_res(g)

        if g is not None:
            st_next(g, 1)
        davp = None
        if it - 2 in deferred:
            # previous head's AV_c1: its recip ran on DVE while this head's
            # first S^T tile kept the PE busy; the ynT multiply is emitted
            # after the q projection so it doesn't delay this head's RoPE
            # multiplies in the DVE stream
            dpT, dri, dynT = deferred.pop(it - 2)
            if it - 2 == NH - 1:
                # the proj psum pair is idle by now; using it lets the AV
                # matmuls overlap the rowsum/recip drain of the final head
                davp = pp.tile([P, 512], F32, tag="proj", name="av1_last")
            else:
                davp = ap_.tile([P, 512], F32, tag="avrs", name=f"av1_{it-2}")
            av_chunk(it - 2, 1, dpT, dri, dynT, davp, emit_ynt=False)
        vps = {0: [(0, 0), (1, 0), (2, 0), (3, 0),
                   (4, 0), (5, 0), (6, 0), (7, 0)],
               1: [(0, 1), (1, 1), (2, 1), (3, 1)],
               2: [(4, 1), (5, 1), (6, 1), (7, 1)]}.get(it, [])
        nvp = 2 if it == 0 else 1
        # at it=0 run the k chunk-0 projection right after q's so the
        # first S^T tile's DVE dependency chain completes sooner
        order = ((0, 0), (1, 0), (0, 1), (1, 1)) if it == 0 else \
                ((0, 0), (0, 1), (1, 0), (1, 1))

        def vp_fill():
            for st_i, c in vps[:nvp]:
                vproj_piece(st_i, c)
            del vps[:nvp]

        if h is not None:
            proj_chunk(h, *order[0])
            vp_fill()
        if davp is not None:
            emit_ynt_mult(1, dri, dynT, davp)
            ynTs[it - 2] = dynT
            if it - 2 == NH - 1:
                emit_outproj(NH - 1, cs=(1,))
                finish_out(1)  # last head: close out chunk 1 right away
        if g is not None:
            st_next(g, 2)
        if h is not None:
            proj_chunk(h, *order[1])
            vp_fill()
        if g is not None:
            st_next(g, 3)
        if h is not None:
            proj_chunk(h, *order[2])
            vp_fill()
        if g is not None:
            st_next(g, 4)
        if 2 <= it and it - 2 < NH - 1:
            emit_outproj(it - 2)  # deferred: ynT computed last iteration
        if h is not None:
            proj_chunk(h, *order[3])
            vp_fill()
        if g is not None:
            st_tile(g, 4, pT)
            rs_ps = ap_.tile([P, 512], F32, tag="avrs", name=f"rs0_{g}")
            rs_chunk(g, 0, pT, ri, [0, 1, 2, 3], rs_ps)
            st_tile(g, 5, pT)
            av_ps = ap_.tile([P, 512], F32, tag="avrs", name=f"av0_{g}")
            av_chunk(g, 0, pT, ri, ynT, av_ps)
            if g == NH - 1:
                ynTs[g] = ynT
                emit_outproj(g, cs=(0,))
                finish_out(0)  # last head: close out chunk 0 early
            st_tile(g, 6, pT)
            rs_ps1 = ap_.tile([P, 512], F32, tag="avrs", name=f"rs1_{g}")
            rs_chunk(g, 1, pT, ri, [0, 1, 2, 3], rs_ps1)
            st_tile(g, 7, pT)
            rs_chunk(g, 1, pT, ri, [4, 5, 6, 7], rs_ps1)
            deferred[g] = (pT, ri, ynT)
            if g > 0:
                qrot.pop(g), krot.pop(g)
                state.pop((g, 0)), state.pop((g, 1))

    ctx.close()


def _rope_tables_np():
    """Bit-faithful replication of reference._rope_tables (float32 jax ops)."""
    import jax.numpy as jnp
    half = E // 2
    dtype = jnp.float32
    angles = jnp.power(jnp.asarray(10000.0, dtype),
                       2.0 * jnp.arange(half, dtype=dtype) / E)
    theta = jnp.arange(S, dtype=dtype)[:, None] * angles[None, :]
    return np.asarray(jnp.cos(theta)), np.asarray(jnp.sin(theta))


def make_in_maps(x, w_q, w_k, w_v, w_o):
    x = np.asarray(x, np.float32)
    w_q = np.asarray(w_q, np.float32)
    w_k = np.asarray(w_k, np.float32)
    w_v = np.asarray(w_v, np.float32)
    w_o = np.asarray(w_o, np.float32)

    def b16(a):
        return np.ascontiguousarray(a).astype(ml_dtypes.bfloat16)

    cos, sin = _rope_tables_np()            # [S, 64] f32
    ropeC = np.repeat(cos.T, 2, axis=0)     # [128, S]
    ropeS = np.repeat(sin.T, 2, axis=0)
    ropeS[0::2] *= -1.0
    # fused per-chunk [C | S] table so one DVE multiply covers both RoPE
    # products of a 512-column chunk
    ropeCS = np.concatenate([ropeC[:, 0:512], ropeS[:, 0:512],
                             ropeC[:, 512:1024], ropeS[:, 512:1024]], axis=1)

    tri = np.where(np.arange(P)[None, :] < np.arange(P)[:, None],
                   np.float32(-1e30), np.float32(0.0))
    idn = np.eye(P, dtype=np.float32)

    perm = np.arange(P)
    perm = perm ^ 1  # swap adjacent pairs

    def blocksT(w, heads, permute=False):
        # w: (2048, 128); heads: list of global head indices
        # -> (128, len*128) with column block j = w[h_j*128:(h_j+1)*128].T
        cols = []
        for hgl in heads:
            blk = w[hgl * P:(hgl + 1) * P, :]
            if permute:
                blk = blk[perm, :]
            cols.append(blk.T)
        return np.concatenate(cols, axis=1)

    in_maps = []
    for core in range(NCORES):
        b = core // 2
        g = core % 2
        heads = [g * NH + j for j in range(NH)]
        woTc = np.concatenate(
            [w_o[:, h * P:(h + 1) * P].T for h in heads], axis=1)
        in_maps.append({
            "xT": b16(x[b].T),
            "wqT": b16(blocksT(w_q, heads)),
            "wqpT": b16(blocksT(w_q, heads, permute=True)),
            "wkT": b16(blocksT(w_k, heads)),
            "wkpT": b16(blocksT(w_k, heads, permute=True)),
            "wvT": b16(blocksT(w_v, heads)),
            "woT": b16(woTc),
            "ropeCS": b16(ropeCS),
            "tri": b16(tri),
            "idn": b16(idn),
            "ones": np.ones((P, P), ml_dtypes.bfloat16),
        })
    return in_maps


_NC_CACHE = {}


def get_nc():
    if "nc" not in _NC_CACHE:
        _NC_CACHE["nc"] = build_bass()
    return _NC_CACHE["nc"]


def run(x, w_q, w_k, w_v, w_o, trace=False, trace_cores=None):
    nc = get_nc()
    in_maps = make_in_maps(x, w_q, w_k, w_v, w_o)
    res = run_bass_kernel_spmd(nc, in_maps, list(range(NCORES)), trace=trace,
                               trace_cores=trace_cores)
    out = np.zeros((B, S, E), np.float32)
    for core in range(NCORES):
        out[core // 2] += res.results[core]["outT"].T
    return out, res


def kernel(x, w_q, w_k, w_v, w_o):
    out, _ = run(x, w_q, w_k, w_v, w_o)
    return out


# revision 40
# speedup vs baseline: 1.0079x; 1.0051x over previous
"""Multi-head attention (16 heads, RoPE, causal) Trainium2 Bass kernel.

Sharding: 8 cores = 4-way data-parallel over batch x 2-way tensor-parallel
over heads (each core: 1 batch, 8 heads). Per-core partial outputs (over its
8 heads) are summed pairwise on the host (the w_o "all-reduce").

v2: all-bf16 datapath. All matmul operands are bf16 (fast weight load, no
f32r small-moving penalty, half the input DMA bytes). The softmax rowsum is
computed by pre-accumulating the 8 P^T k-tiles on the Vector engine (bf16
SBUF adds run 2 elem/cycle) into one [128, S] tile, then a single ones-
matmul per head reduces over partitions -- replacing 4608 PE matmul columns
per head with 1024. Elementwise work (RoPE multiplies/adds, normalization,
reciprocal, v copies) is statically balanced across Vector / GpSimd /
Scalar so no engine exceeds the PE's matmul stream.

Per-core algorithm (S=1024, E=128 = head dim, 8 local heads):
  - xT [e, s] bf16; per-head wT [e, d] blocks give qT/kT in [d, s] layout.
  - RoPE: rot(q)T = ropeC (.) qT + ropeS (.) (perm q)T with perm via
    pair-swapped weight copies (extra projection matmuls).
  - S^T[k, q] blocks per 128-wide k tile; causal diagonal gets a -1e30
    upper-triangular bias via one bf16 idn x tri matmul into the same PSUM
    accumulation group; Scalar applies exp(scale*x) writing P^T bf16.
  - rowsums: DVE pre-sum of P^T tiles + one ones-matmul; fast reciprocal.
  - y^T[d, q] = sum_j v_j @ P^T_j, normalized by recip rowsums, then
    out^T[e, s] += woT_h.T @ ynT_h accumulated in PSUM across heads.
"""

import os
import sys

import ml_dtypes
import numpy as np

for _p in ("/opt/trn_rl_repo",):
    if os.path.isdir(_p) and _p not in sys.path:
        sys.path.append(_p)

import concourse.bass as bass  # noqa: E402
import concourse.tile as tile  # noqa: E402
from concourse import bacc, mybir  # noqa: E402
from concourse.bass_utils import run_bass_kernel_spmd  # noqa: E402

F32 = mybir.dt.float32
BF16 = mybir.dt.bfloat16

B, S, E, H = 4, 1024, 128, 16
NCORES = 8
NH = 8          # heads per core
P = 128
NT = S // P     # 8 seq tiles
SCALE = 1.0 / float(np.sqrt(np.float32(E)))
Exp = mybir.ActivationFunctionType.Exp
MULT = mybir.AluOpType.mult
ADD = mybir.AluOpType.add


def build_bass():
    nc = bacc.Bacc("TRN2", target_bir_lowering=False, debug=False,
                   num_devices=NCORES)

    def din(name, shape, dt=BF16):
        return nc.dram_tensor(name, shape, dt, kind="ExternalInput").ap()

    xT = din("xT", [P, S])
    wqT = din("wqT", [P, NH * P])
    wqpT = din("wqpT", [P, NH * P])
    wkT = din("wkT", [P, NH * P])
    wkpT = din("wkpT", [P, NH * P])
    wvT = din("wvT", [P, NH * P])
    woT = din("woT", [P, NH * P])
    ropeCS = din("ropeCS", [P, 2 * S])
    tri = din("tri", [P, P])
    idn = din("idn", [P, P])
    ones = din("ones", [P, P])
    outT = nc.dram_tensor("outT", [P, S], F32, kind="ExternalOutput").ap()

    with tile.TileContext(nc) as tc:
        _build(tc, xT, wqT, wqpT, wkT, wkpT, wvT, woT, ropeCS, tri,
               idn, ones, outT)
    nc.compile()
    return nc


def _build(tc, xT, wqT, wqpT, wkT, wkpT, wvT, woT, ropeCS, tri, idn,
           ones, outT):
    nc = tc.nc

    from contextlib import ExitStack
    ctx = ExitStack()
    const = ctx.enter_context(tc.tile_pool(name="const", bufs=1))
    vpool = ctx.enter_context(tc.tile_pool(name="vpool", bufs=1))
    ppool = ctx.enter_context(tc.tile_pool(name="ppool", bufs=2))
    qkpool = ctx.enter_context(tc.tile_pool(name="qkpool", bufs=2))
    tmppool = ctx.enter_context(tc.tile_pool(name="tmppool", bufs=2))
    npool = ctx.enter_context(tc.tile_pool(name="npool", bufs=2))
    opool = ctx.enter_context(tc.tile_pool(name="opool", bufs=1))
    # PSUM budget (8 banks): proj 2-bank pair, S^T ring 3, one rotating
    # bank for the rowsum/AV accumulation groups, outproj accumulators 2.
    pp = ctx.enter_context(tc.tile_pool(name="pp", bufs=1, space="PSUM"))
    sp = ctx.enter_context(tc.tile_pool(name="sp", bufs=3, space="PSUM"))
    ap_ = ctx.enter_context(tc.tile_pool(name="ap", bufs=1, space="PSUM"))
    op = ctx.enter_context(tc.tile_pool(name="op", bufs=2, space="PSUM"))

    # Constants into SBUF, first-use order, each large tensor split into
    # two half-loads so completion semaphores fire as early as possible
    # (the warmup chain q_c0 -> ropeCS multiply -> q_c1 is gated on them).
    halves = {}

    def load(pool, ap, shape, tag):
        t = pool.tile(shape, BF16, tag=tag)
        halves[tag] = (t, ap)
        return t

    def load_half(tag, hf):
        t, ap = halves[tag]
        w = t.shape[-1] // 2
        sl = slice(hf * w, (hf + 1) * w)
        nc.sync.dma_start(t[:, sl], ap[:, sl])

    xT_sb = load(const, xT, [P, S], "xT")
    wqT_sb = load(const, wqT, [P, NH * P], "wqT")
    wqpT_sb = load(const, wqpT, [P, NH * P], "wqpT")
    ropeCS_sb = load(const, ropeCS, [P, 2 * S], "ropeCS")
    wkT_sb = load(const, wkT, [P, NH * P], "wkT")
    wkpT_sb = load(const, wkpT, [P, NH * P], "wkpT")
    wvT_sb = load(const, wvT, [P, NH * P], "wvT")
    woT_sb = load(const, woT, [P, NH * P], "woT")
    for tag in ("xT", "wqT", "wqpT", "ropeCS", "wvT", "wkT", "wkpT"):
        load_half(tag, 0)
    for tag in ("xT", "wqT", "wqpT", "ropeCS", "wvT", "wkT", "wkpT"):
        load_half(tag, 1)
    tri_sb = const.tile([P, P], BF16, tag="tri")
    nc.sync.dma_start(tri_sb[:], tri)
    idn_sb = const.tile([P, P], BF16, tag="idn")
    nc.sync.dma_start(idn_sb[:], idn)
    ones_sb = const.tile([P, P], BF16, tag="ones")
    nc.sync.dma_start(ones_sb[:], ones)
    load_half("woT", 0), load_half("woT", 1)

    # v for all heads, [s_in_tile, s_tile, head*128+d]
    v_sb = vpool.tile([P, NT, NH * P], BF16, tag="v")

    # Warm up the PE clock while the input DMAs stream: HAM only grants
    # full clock after ~3us of continuous busy, so a dozen dummy matmuls
    # on a memset tile let the first real matmuls run at 2.4GHz instead
    # of ramping through them.
    warm = opool.tile([P, 512], BF16, tag="warm")
    nc.gpsimd.memset(warm[:], 0.0)
    for i in range(16):
        wps = sp.tile([P, 512], F32, tag="att", name=f"warm{i}")
        nc.tensor.matmul(wps[:], warm[:, 0:P], warm[:], start=True, stop=True)

    def vproj_piece(st_i, c):
        """One (s-tile, head-half) piece of the V projection. Pieces are
        spread over the first three iterations in need-order: the c==0
        half (heads 0-3) lands during it=0, c==1 trickles in behind it,
        so the Scalar engine's copies never delay the first head's exps."""
        vp = sp.tile([P, 512], F32, tag="att", name=f"vp{st_i}_{c}")
        nc.tensor.matmul(vp[:], xT_sb[:, st_i * P:(st_i + 1) * P],
                         wvT_sb[:, c * 512:(c + 1) * 512],
                         start=True, stop=True)
        nc.scalar.copy(v_sb[:, st_i, c * 512:(c + 1) * 512], vp[:])

    # persistent output accumulator psum (2 banks)
    out_ps = [op.tile([P, 512], F32, tag="out", name=f"out_ps{c}")
              for c in range(2)]

    qrot = {}
    krot = {}
    ynTs = {}
    state = {}

    def proj_chunk(h, qk, c):
        """One 512-chunk of head h's q/qp (qk=0) or k/kp (qk=1) projection.
        Both matmuls land in one 2-bank PSUM pair so a single DVE multiply
        against the fused [C|S] table produces both RoPE products; the
        per-chunk add (bf16 SBUF, GpSimd) completes rot[:, chunk] early so
        the next head's S^T tiles are not gated on the full-row rotation."""
        if (h, qk) not in state:
            dst = qkpool.tile([P, S], BF16, tag=("qrot", "krot")[qk],
                              name=f"rot{h}_{qk}")
            state[(h, qk)] = dst
            (qrot, krot)[qk][h] = dst
        dst = state[(h, qk)]
        wt = (wqT_sb, wkT_sb)[qk][:, h * P:(h + 1) * P]
        wpt = (wqpT_sb, wkpT_sb)[qk][:, h * P:(h + 1) * P]
        sl = slice(c * 512, (c + 1) * 512)
        ab = pp.tile([P, 1024], F32, tag="proj", name=f"pab{h}_{qk}_{c}")
        nc.tensor.matmul(ab[:, 0:512], wt, xT_sb[:, sl], start=True, stop=True)
        nc.tensor.matmul(ab[:, 512:1024], wpt, xT_sb[:, sl],
                         start=True, stop=True)
        cs = tmppool.tile([P, 1024], BF16, tag=f"tmp{qk}",
                          name=f"cs{h}_{qk}_{c}")
        nc.vector.tensor_tensor(cs[:], ab[:], ropeCS_sb[:, c * 1024:
                                                        (c + 1) * 1024], MULT)
        eng = nc.vector if h == 0 else nc.gpsimd
        eng.tensor_tensor(dst[:, sl], cs[:, 0:512], cs[:, 512:1024], ADD)

    def st_tile(g, j, pT):
        """S^T block j for head g + exp."""
        qr, kr = qrot[g], krot[g]
        kblk = kr[:, j * P:(j + 1) * P]
        chunks = [(j * P, 512), (512, 1024)] if j < 4 else [(j * P, 1024)]
        # the last head runs with no projection work interleaved; borrow
        # the idle proj psum pair to deepen its S^T ring
        xtra = (g == NH - 1 and j >= 4 and j % 2 == 0)
        for ci, (a, bnd) in enumerate(chunks):
            w = bnd - a
            if xtra:
                stt = pp.tile([P, 512], F32, tag="proj", name=f"sx{g}_{j}")
            else:
                stt = sp.tile([P, 512], F32, tag="att")
            diag = (ci == 0)
            nc.tensor.matmul(stt[:, :w], kblk, qr[:, a:bnd],
                             start=True, stop=not diag)
            if diag:
                nc.tensor.matmul(stt[:, :P], idn_sb[:], tri_sb[:],
                                 start=False, stop=True)
            nc.scalar.activation(pT[:, j, a:bnd], stt[:, :w], Exp,
                                 scale=SCALE)

    def rs_chunk(g, c, pT, ri, jrange, ps):
        """Part of the rowsum accumulation for chunk c (ones-matmul gives
        the k-sum pre-broadcast across partitions)."""
        jmax = 4 * c + 3
        for j in jrange:
            r0 = max(c * 512, j * P)
            r1 = (c + 1) * 512
            nc.tensor.matmul(ps[:, r0 - c * 512:r1 - c * 512],
                             ones_sb[:], pT[:, j, r0:r1],
                             start=(j == 0), stop=(j == jmax))
        if jrange[-1] == jmax:
            nc.vector.reciprocal_approx_fast(ri[:, c * 512:(c + 1) * 512],
                                             ps[:, :512])

    def av_chunk(g, c, pT, ri, ynT, ps, emit_ynt=True):
        """AV accumulation + normalization for chunk c."""
        jmax = 4 * c + 3
        for j in range(jmax + 1):
            r0 = max(c * 512, j * P)
            r1 = (c + 1) * 512
            nc.tensor.matmul(ps[:, r0 - c * 512:r1 - c * 512],
                             v_sb[:, j, g * P:(g + 1) * P],
                             pT[:, j, r0:r1],
                             start=(j == 0), stop=(j == jmax))
        if emit_ynt:
            emit_ynt_mult(c, ri, ynT, ps)

    def emit_ynt_mult(c, ri, ynT, ps):
        nc.vector.tensor_tensor(ynT[:, c * 512:(c + 1) * 512],
                                ps[:, :512],
                                ri[:, c * 512:(c + 1) * 512], MULT)

    def emit_outproj(g, cs=(0, 1)):
        ynT = ynTs[g]
        for c in cs:
            nc.tensor.matmul(out_ps[c][:], woT_sb[:, g * P:(g + 1) * P],
                             ynT[:, c * 512:(c + 1) * 512],
                             start=(g == 0), stop=(g == NH - 1))

    # Software-pipelined head loop. Head h's projection matmuls and head
    # g=h-1's S^T / rowsum / AV matmuls are interleaved instruction by
    # instruction so the PE always has independent work queued while the
    # Scalar engine drains exps (S^T tiles are paced by the 3-buffer PSUM
    # ring) -- idle PE gaps trigger HAM clock throttling, which is worth
    # more than the gaps themselves. The rowsum/AV accumulation groups
    # rotate through one dedicated PSUM bank: rs_c0 -> av_c0 -> rs_c1 ->
    # av_c1, each WAR-dependency hidden behind interleaved S^T work.
    out_sb = opool.tile([P, S], F32, tag="osb")

    def finish_out(c):
        sl = slice(c * 512, (c + 1) * 512)
        nc.scalar.copy(out_sb[:, sl], out_ps[c][:])
        nc.sync.dma_start(outT[:, sl], out_sb[:, sl])

    res = {}
    st_done = {}

    def get_res(g):
        if g not in res:
            pT = ppool.tile([P, NT, S], BF16, tag="pT", name=f"pT{g}")
            ri = npool.tile([P, S], F32, tag="ri", name=f"ri{g}")
            ynT = npool.tile([P, S], BF16, tag="ynT", name=f"ynT{g}")
            res[g] = (pT, ri, ynT)
            st_done[g] = 0
        return res[g]

    def st_next(g, upto):
        pT = get_res(g)[0]
        while st_done[g] < upto:
            st_tile(g, st_done[g], pT)
            st_done[g] += 1

    deferred = {}
    for it in range(NH + 2):
        h = it if it < NH else None
        g = it - 1 if 1 <= it <= NH else None
        if g is not None:
            pT, ri, ynT = get_res(g)

        if g is not None:
            st_next(g, 1)
        davp = None
        if it - 2 in deferred:
            # previous head's AV_c1: its recip ran on DVE while this head's
            # first S^T tile kept the PE busy; the ynT multiply is emitted
            # after the q projection so it doesn't delay this head's RoPE
            # multiplies in the DVE stream
            dpT, dri, dynT = deferred.pop(it - 2)
            if it - 2 == NH - 1:
                # the proj psum pair is idle by now; using it lets the AV
                # matmuls overlap the rowsum/recip drain of the final head
                davp = pp.tile([P, 512], F32, tag="proj", name="av1_last")
            else:
                davp = ap_.tile([P, 512], F32, tag="avrs", name=f"av1_{it-2}")
            av_chunk(it - 2, 1, dpT, dri, dynT, davp, emit_ynt=False)
        vps = {0: [(0, 0), (1, 0), (2, 0), (3, 0),
                   (4, 0), (5, 0), (6, 0), (7, 0)],
               1: [(0, 1), (1, 1), (2, 1), (3, 1)],
               2: [(4, 1), (5, 1), (6, 1), (7, 1)]}.get(it, [])
        nvp = 2 if it == 0 else 1
        # at it=0 run the k chunk-0 projection right after q's so the
        # first S^T tile's DVE dependency chain completes sooner
        order = ((0, 0), (1, 0), (0, 1), (1, 1)) if it == 0 else \
                ((0, 0), (0, 1), (1, 0), (1, 1))

        def vp_fill():
            for st_i, c in vps[:nvp]:
                vproj_piece(st_i, c)
            del vps[:nvp]

        if h is not None:
            proj_chunk(h, *order[0])
            vp_fill()
        if davp is not None:
            emit_ynt_mult(1, dri, dynT, davp)
            ynTs[it - 2] = dynT
            if it - 2 == NH - 1:
                emit_outproj(NH - 1, cs=(1,))
                finish_out(1)  # last head: close out chunk 1 right away
        if g is not None:
            st_next(g, 2)
        if h is not None:
            proj_chunk(h, *order[1])
            vp_fill()
        if g is not None:
            st_next(g, 3)
        if h is not None:
            proj_chunk(h, *order[2])
            vp_fill()
        if g is not None:
            st_next(g, 4)
        if 2 <= it and it - 2 < NH - 1:
            emit_outproj(it - 2)  # deferred: ynT computed last iteration
        if h is not None:
            proj_chunk(h, *order[3])
            vp_fill()
        if g is not None:
            st_tile(g, 4, pT)
            rs_ps = ap_.tile([P, 512], F32, tag="avrs", name=f"rs0_{g}")
            rs_chunk(g, 0, pT, ri, [0, 1, 2, 3], rs_ps)
            st_tile(g, 5, pT)
            av_ps = ap_.tile([P, 512], F32, tag="avrs", name=f"av0_{g}")
            av_chunk(g, 0, pT, ri, ynT, av_ps)
            if g == NH - 1:
                ynTs[g] = ynT
                emit_outproj(g, cs=(0,))
                finish_out(0)  # last head: close out chunk 0 early
            st_tile(g, 6, pT)
            rs_ps1 = ap_.tile([P, 512], F32, tag="avrs", name=f"rs1_{g}")
            rs_chunk(g, 1, pT, ri, [0, 1, 2, 3], rs_ps1)
            st_tile(g, 7, pT)
            rs_chunk(g, 1, pT, ri, [4, 5, 6, 7], rs_ps1)
            deferred[g] = (pT, ri, ynT)
            if g > 0:
                qrot.pop(g), krot.pop(g)
                state.pop((g, 0)), state.pop((g, 1))

    ctx.close()


def _rope_tables_np():
    """Bit-faithful replication of reference._rope_tables (float32 jax ops)."""
    import jax.numpy as jnp
    half = E // 2
    dtype = jnp.float32
    angles = jnp.power(jnp.asarray(10000.0, dtype),
                       2.0 * jnp.arange(half, dtype=dtype) / E)
    theta = jnp.arange(S, dtype=dtype)[:, None] * angles[None, :]
    return np.asarray(jnp.cos(theta)), np.asarray(jnp.sin(theta))


def make_in_maps(x, w_q, w_k, w_v, w_o):
    x = np.asarray(x, np.float32)
    w_q = np.asarray(w_q, np.float32)
    w_k = np.asarray(w_k, np.float32)
    w_v = np.asarray(w_v, np.float32)
    w_o = np.asarray(w_o, np.float32)

    def b16(a):
        return np.ascontiguousarray(a).astype(ml_dtypes.bfloat16)

    cos, sin = _rope_tables_np()            # [S, 64] f32
    ropeC = np.repeat(cos.T, 2, axis=0)     # [128, S]
    ropeS = np.repeat(sin.T, 2, axis=0)
    ropeS[0::2] *= -1.0
    # fused per-chunk [C | S] table so one DVE multiply covers both RoPE
    # products of a 512-column chunk
    ropeCS = np.concatenate([ropeC[:, 0:512], ropeS[:, 0:512],
                             ropeC[:, 512:1024], ropeS[:, 512:1024]], axis=1)

    tri = np.where(np.arange(P)[None, :] < np.arange(P)[:, None],
                   np.float32(-1e30), np.float32(0.0))
    idn = np.eye(P, dtype=np.float32)

    perm = np.arange(P)
    perm = perm ^ 1  # swap adjacent pairs

    def blocksT(w, heads, permute=False):
        # w: (2048, 128); heads: list of global head indices
        # -> (128, len*128) with column block j = w[h_j*128:(h_j+1)*128].T
        cols = []
        for hgl in heads:
            blk = w[hgl * P:(hgl + 1) * P, :]
            if permute:
                blk = blk[perm, :]
            cols.append(blk.T)
        return np.concatenate(cols, axis=1)

    in_maps = []
    for core in range(NCORES):
        b = core // 2
        g = core % 2
        heads = [g * NH + j for j in range(NH)]
        woTc = np.concatenate(
            [w_o[:, h * P:(h + 1) * P].T for h in heads], axis=1)
        in_maps.append({
            "xT": b16(x[b].T),
            "wqT": b16(blocksT(w_q, heads)),
            "wqpT": b16(blocksT(w_q, heads, permute=True)),
            "wkT": b16(blocksT(w_k, heads)),
            "wkpT": b16(blocksT(w_k, heads, permute=True)),
            "wvT": b16(blocksT(w_v, heads)),
            "woT": b16(woTc),
            "ropeCS": b16(ropeCS),
            "tri": b16(tri),
            "idn": b16(idn),
            "ones": np.ones((P, P), ml_dtypes.bfloat16),
        })
    return in_maps


_NC_CACHE = {}


def get_nc():
    if "nc" not in _NC_CACHE:
        _NC_CACHE["nc"] = build_bass()
    return _NC_CACHE["nc"]


def run(x, w_q, w_k, w_v, w_o, trace=False, trace_cores=None):
    nc = get_nc()
    in_maps = make_in_maps(x, w_q, w_k, w_v, w_o)
    res = run_bass_kernel_spmd(nc, in_maps, list(range(NCORES)), trace=trace,
                               trace_cores=trace_cores)
    out = np.zeros((B, S, E), np.float32)
    for core in range(NCORES):
        out[core // 2] += res.results[core]["outT"].T
    return out, res


def kernel(x, w_q, w_k, w_v, w_o):
    out, _ = run(x, w_q, w_k, w_v, w_o)
    return out
